# revision 19
# baseline (speedup 1.0000x reference)
"""Differential multi-head attention on 8 TRN2 NeuronCores (Bass/Tile).

Problem (hardcoded): B=2, T=N=2048, HID=1024, H=16 heads, DH=64, HALF=32,
DEPTH=6, causal. Reference:
    q = query @ Wq.T ; k = key_ @ Wk.T ; v = value @ Wv.T
    2H half-heads of size 32; att = softmax(causal(q k^T / sqrt(32)))
    att = att_half1 - lambda_full * att_half2        (per head)
    out = att @ v ; RMSNorm(head dim) * g * (1-lambda_init) ; out @ Wo.T

Sharding: batch*heads across 8 cores. Core c handles batch b=c//4 and 4
heads hs=4*(c%4)..hs+4. Host sums the 4 per-core partial Wo projections of
each batch.

v3 schedule: one software pipeline ordered so the PE never starves.
  - projections stream per 512-column block (single batched DMA per x
    block), pair-0 attention follows each block; pair-1 attention runs
    with pair-0/pair-1 stats drip-fed behind it; output projection last.
  - attention emission is software-pipelined: the AV matmuls for step nu
    are emitted two steps behind the QK matmuls, so the in-order PE queue
    never waits on the Exp that links them.
  - causal masking is applied IN PSUM by a "ramp" matmul on the diagonal
    128x128 block (A[d,n]=8*[d<n], B[d,t]=-8*[d>=t] adds -64*(n-t) to
    masked scores, so Exp gives exact 0). One accumulation group per PSUM
    bank: the full-width qk matmul opens it, the ramp matmul closes it.
  - stats use NO DRAM bounces: RMSNorm is scale-invariant, so instead of
    broadcasting 1/l1 and lambda/l2 we broadcast one row
    r = (lambda*kappa*l1)/l2 with gpsimd.partition_broadcast, compute
    od = kappa*o1 - r*o2, and get the per-token mean-square with
    gpsimd.partition_all_reduce (Pool engine, no DMA round trips).
    kappa=1/4 keeps od and od^2 (f32) in range; the RMS rescale absorbs it.
  - PSUM: "sqk" (proj + y accumulators) 2x1 bank, "sc" (scores) 2x2
    banks, "oav" (attention out + denominator row) 1x2 banks = 8 banks.
All matmul operands fp16 (fp32 PSUM accumulation); scale 1/sqrt(32) folded
into Wq, g*(1-lambda_init) folded into Wo.
"""

import math
from contextlib import ExitStack

import numpy as np

import concourse.bass as bass
import concourse.tile as tile
from concourse import bacc, bass_isa, mybir
from concourse.bass_utils import run_bass_kernel_spmd

# Prefer the combined ln+exp ACT table set so softmax Exp and RMSNorm Ln/Exp
# share one set (a set switch costs ~2.7us on ScalarE).
import concourse.hw_specs as _hw_specs
_orig_get_tables = _hw_specs.get_activation_tables
def _tables_ln_exp_first(arch):
    tabs = _orig_get_tables(arch)
    if "natural_log_exp_and_others" not in tabs:
        return tabs
    return {
        k: (set() if k in ("exp_and_others", "natural_log", "exp_and_friends")
            else v)
        for k, v in tabs.items()
    }
_hw_specs.get_activation_tables = _tables_ln_exp_first
bacc.get_activation_tables = _tables_ln_exp_first

dt = mybir.dt
AF = mybir.ActivationFunctionType
ALU = mybir.AluOpType

B, T, NN, HID = 2, 2048, 2048, 1024
H, DH, HALF = 16, 64, 32
DEPTH = 6
LAMBDA_INIT = 0.8 - 0.6 * math.exp(-0.3 * DEPTH)
EPS = 1e-5
N_CORES = 8
HPC = 4          # heads per core
KAPPA = 0.25     # pre-RMS scale guard (absorbed by the RMS rescale)
LN_BIAS = 1e-10  # Ln(0) guard; equivalent eps ~ 1e-5/l1^2 ~ 0

_CACHE = {}


def _build(lam: float, reps: int = 1):
    nc = bacc.Bacc(
        "TRN2", target_bir_lowering=False, debug=False, num_devices=N_CORES
    )

    f16, f32 = dt.float16, dt.float32

    xq_d = nc.dram_tensor("xq", [HID, T], f16, kind="ExternalInput").ap()
    xk_d = nc.dram_tensor("xk", [HID, T], f16, kind="ExternalInput").ap()
    xv_d = nc.dram_tensor("xv", [HID, T], f16, kind="ExternalInput").ap()
    wq_d = nc.dram_tensor("wq", [HID, 256], f16, kind="ExternalInput").ap()
    wk_d = nc.dram_tensor("wk", [HID, 256], f16, kind="ExternalInput").ap()
    wv_d = nc.dram_tensor("wv", [HID, 256], f16, kind="ExternalInput").ap()
    wo_d = nc.dram_tensor("wo", [256, HID], f16, kind="ExternalInput").ap()
    ma_d = nc.dram_tensor("ma", [128, 128], f16, kind="ExternalInput").ap()
    mb_d = nc.dram_tensor("mb", [128, 128], f16, kind="ExternalInput").ap()
    y_d = nc.dram_tensor("y", [T, HID], f16, kind="ExternalOutput").ap()

    with tile.TileContext(nc) as tc, ExitStack() as ctx:
        ctx.enter_context(
            nc.allow_low_precision(reason="fp16 attention pipeline by design")
        )
        consts = ctx.enter_context(tc.tile_pool(name="consts", bufs=1))
        xpool = ctx.enter_context(tc.tile_pool(name="xpool", bufs=2))
        qkpool = ctx.enter_context(tc.tile_pool(name="qkpool", bufs=1))
        vpool = ctx.enter_context(tc.tile_pool(name="vpool", bufs=1))
        ppool = ctx.enter_context(tc.tile_pool(name="ppool", bufs=4))
        opool = ctx.enter_context(tc.tile_pool(name="opool", bufs=1))
        npool = ctx.enter_context(tc.tile_pool(name="npool", bufs=1))
        spool = ctx.enter_context(tc.tile_pool(name="spool", bufs=2))
        ypool = ctx.enter_context(tc.tile_pool(name="ypool", bufs=4))
        psum = ctx.enter_context(tc.tile_pool(name="psum", bufs=1, space="PSUM"))

        def sqk_tile(name):
            return psum.tile([128, 512], f32, tag="sqk", bufs=2, name=name)

        for _rep in range(reps):
            # ---------------- constants / persistent tiles ----------------
            wq_s = consts.tile([128, 8, 256], f16, tag="wq")
            wk_s = consts.tile([128, 8, 256], f16, tag="wk")
            wv_s = consts.tile([128, 8, 256], f16, tag="wv")
            wo_s = consts.tile([128, 2, HID], f16, tag="wo")
            ma_s = consts.tile([128, 128], f16, tag="ma")
            mb_s = consts.tile([128, 128], f16, tag="mb")
            ebias = consts.tile([128, 1], f32, tag="ebias")

            qT = [qkpool.tile([128, T], f16, tag=f"qT{pp}", name=f"qT{pp}")
                  for pp in range(2)]
            kT = [qkpool.tile([128, T], f16, tag=f"kT{pp}", name=f"kT{pp}")
                  for pp in range(2)]
            v_s = [vpool.tile([128, 16, 2, 65], f16, tag=f"v{pp}", name=f"v{pp}")
                   for pp in range(2)]
            # (65, g, s, c, t) attention output + denominator row
            o_s = [opool.tile([65, 2, 2, 4, 512], f16, tag=f"o_{pp}",
                              name=f"o_{pp}") for pp in range(2)]
            # per-c-block tiles so y chunks only depend on their own block
            o_norm = {(pp, c): npool.tile([128, 512], f16, tag=f"on{pp}c{c}",
                                          name=f"on{pp}c{c}")
                      for pp in range(2) for c in range(4)}

            def load_xblock(src, c, tag):
                xt = xpool.tile([128, 8, 512], f16, tag=tag, name=f"{tag}{c}")
                nc.sync.dma_start(
                    out=xt,
                    in_=src[:, 512 * c : 512 * c + 512].rearrange(
                        "(d p) t -> p d t", p=128),
                )
                return xt

            # startup: interleave weight loads with first x blocks
            nc.sync.dma_start(out=wq_s, in_=wq_d.rearrange("(d p) j -> p d j", p=128))
            xq0 = load_xblock(xq_d, 0, "xq")
            nc.sync.dma_start(out=wk_s, in_=wk_d.rearrange("(d p) j -> p d j", p=128))
            xk0 = load_xblock(xk_d, 0, "xk")
            nc.sync.dma_start(out=wv_s, in_=wv_d.rearrange("(d p) j -> p d j", p=128))
            xv0 = load_xblock(xv_d, 0, "xv")
            nc.sync.dma_start(out=ma_s, in_=ma_d)
            nc.sync.dma_start(out=mb_s, in_=mb_d)
            nc.sync.dma_start(out=wo_s, in_=wo_d.rearrange("(k p) e -> p k e", p=128))
            nc.vector.memset(ebias, LN_BIAS)
            nc.vector.memset(v_s[0][:, :, :, 64:65], 1.0)
            nc.vector.memset(v_s[1][:, :, :, 64:65], 1.0)

            def qk_block(xt, w_s, dst, pp, c):
                acc = sqk_tile("acc")
                for d in range(8):
                    nc.tensor.matmul(
                        acc,
                        w_s[:, d, 128 * pp : 128 * pp + 128],
                        xt[:, d, :],
                        start=(d == 0),
                        stop=(d == 7),
                    )
                nc.vector.tensor_copy(dst[pp][:, 512 * c : 512 * c + 512], acc)

            def v_block(xt, c):
                for nl in range(4):
                    nu = 4 * c + nl
                    acc = sqk_tile("accv")[:, 0:256]
                    for d in range(8):
                        nc.tensor.matmul(
                            acc,
                            xt[:, d, 128 * nl : 128 * nl + 128],
                            wv_s[:, d, :],
                            start=(d == 0),
                            stop=(d == 7),
                        )
                    for pp in range(2):
                        nc.vector.tensor_copy(
                            v_s[pp][:, nu, :, 0:64],
                            acc[:, 128 * pp : 128 * pp + 128].rearrange(
                                "p (g j) -> p g j", g=2
                            ),
                        )

            # ------------- attention (software-pipelined emission) -------------
            pend = []   # deferred AV closures, emitted 2 qk-steps behind

            def drain(to):
                while len(pend) > to:
                    pend.pop(0)()

            def attn_group(pp, c, g):
                last = 4 * c + 3
                po = psum.tile([65, 2, 512], f32, tag="oav", bufs=1, name="po")
                for nu in range(last + 1):
                    diag = nu >= 4 * c
                    lo = 128 * (nu - 4 * c) if diag else 0
                    S = psum.tile([128, 2, 512], f32, tag="sc", bufs=2,
                                  name="S")
                    for s in range(2):
                        hh = 2 * g + s
                        kk = kT[pp][32 * hh : 32 * hh + 32,
                                    128 * nu : 128 * nu + 128]
                        if diag:
                            nc.tensor.matmul(
                                S[:, s, lo:512],
                                kk,
                                qT[pp][32 * hh : 32 * hh + 32,
                                       512 * c + lo : 512 * c + 512],
                                start=True, stop=False,
                                tile_position=(32 * hh, 0),
                            )
                            nc.tensor.matmul(
                                S[:, s, lo : lo + 128],
                                ma_s,
                                mb_s,
                                start=False, stop=True,
                                tile_position=(0, 0),
                            )
                        else:
                            nc.tensor.matmul(
                                S[:, s, :],
                                kk,
                                qT[pp][32 * hh : 32 * hh + 32,
                                       512 * c : 512 * c + 512],
                                start=True, stop=True,
                                tile_position=(32 * hh, 0),
                            )
                    pt = ppool.tile([128, 2, 512], f16, tag="pt", name="pt")
                    nc.scalar.activation(
                        out=pt[:, :, lo:512], in_=S[:, :, lo:512], func=AF.Exp,
                    )

                    def av_op(nu=nu, lo=lo, pt=pt, po=po, pp=pp, c=c, g=g,
                              first=(nu == 0), stop=(nu == last)):
                        for s in range(2):
                            nc.tensor.matmul(
                                po[:, s, lo:512],
                                v_s[pp][:, nu, g, :],
                                pt[:, s, lo:512],
                                start=first,
                                stop=stop,
                            )
                            if stop:
                                nc.vector.tensor_copy(
                                    o_s[pp][:, g, s, c, :], po[:, s, :]
                                )

                    pend.append(av_op)
                    drain(2)

            # ------------- per-head stats (no DRAM bounces) -------------
            lk = lam * KAPPA

            def stats_front(pp, g, cs):
                # cs: list of c blocks; returns (od, sq) tiles (flat [64, X])
                w = 512 * len(cs)
                c0 = cs[0]
                l1 = o_s[pp][64:65, g, 0, c0 : c0 + len(cs), :].rearrange(
                    "p c t -> p (c t)")
                l2 = o_s[pp][64:65, g, 1, c0 : c0 + len(cs), :].rearrange(
                    "p c t -> p (c t)")
                o1 = o_s[pp][0:64, g, 0, c0 : c0 + len(cs), :].rearrange(
                    "p c t -> p (c t)")
                o2 = o_s[pp][0:64, g, 1, c0 : c0 + len(cs), :].rearrange(
                    "p c t -> p (c t)")
                # l rows live on partition 64; keep the row math there so
                # base partitions match, then broadcast from partition 64
                ls = spool.tile([65, 2, w], f16, tag="lsc", name="lsc", bufs=2)
                r2 = ls[64:65, 0, :]
                rrow = ls[64:65, 1, :]
                nc.vector.reciprocal(r2, l2)
                nc.vector.scalar_tensor_tensor(
                    out=rrow, in0=l1, scalar=lk, in1=r2,
                    op0=ALU.mult, op1=ALU.mult,
                )
                # partition_broadcast reads absolute partition 0, so hop the
                # row down first (DVE copies can cross partitions)
                r0 = spool.tile([1, w], f16, tag="r0", name="r0", bufs=2)
                nc.vector.tensor_copy(r0, rrow)
                rB = spool.tile([64, w], f16, tag="rB", name="rB", bufs=2)
                nc.gpsimd.partition_broadcast(rB, r0)
                od = spool.tile([64, w], f16, tag="od", name="od", bufs=5)
                m2 = spool.tile([64, w], f16, tag="m2", name="m2", bufs=2)
                nc.vector.tensor_scalar_mul(od, o1, KAPPA)
                nc.vector.tensor_mul(m2, o2, rB)
                nc.vector.tensor_sub(od, od, m2)
                sq = spool.tile([64, w], f32, tag="sq", name="sq", bufs=5)
                nc.vector.tensor_mul(sq, od, od)
                return od, sq

            def stats_back(pp, g, cs, od, sq):
                w = 512 * len(cs)
                c0 = cs[0]
                ms = spool.tile([64, w], f32, tag="ms", name="ms", bufs=2)
                nc.gpsimd.partition_all_reduce(ms, sq, 64, bass_isa.ReduceOp.add)
                sr = spool.tile([64, w], f16, tag="sr", name="sr", bufs=2)
                nc.scalar.activation(out=sr, in_=ms, func=AF.Ln,
                                     scale=1.0 / DH, bias=ebias[0:64, :])
                nc.scalar.activation(out=sr, in_=sr, func=AF.Exp, scale=-0.5)
                if g == 0:
                    nc.vector.tensor_mul(o_norm[(pp, c0)][0:64, :], od, sr)
                else:
                    onh = spool.tile([64, w], f16, tag="onh", name="onh",
                                     bufs=2)
                    nc.vector.tensor_mul(onh, od, sr)
                    nc.sync.dma_start(
                        out=o_norm[(pp, c0)][64:128, :], in_=onh,
                    )

            # ---------------- the pipeline ----------------
            # stats schedule: F two groups after a head's attention group,
            # B two groups after that — the Ln/Exp then never stalls the
            # ACT exp stream, and chains overlap attention of later groups
            sched = {}
            tail_sched = []
            for cc in range(4):
                for pp in range(2):
                    for gg in range(2):
                        i = 4 * cc + 2 * pp + gg
                        for off, op in ((2, "F"), (4, "B")):
                            if i + off < 16:
                                sched.setdefault(i + off, []).append(
                                    (op, pp, gg, cc))
                            else:
                                tail_sched.append((i + off, op, pp, gg, cc))
            tail_sched = [t[1:] for t in sorted(tail_sched)]
            fr = {}

            def run_sched(ops):
                for op, pp, g, cc in ops:
                    if op == "F":
                        fr[(pp, g, cc)] = stats_front(pp, g, [cc])
                    else:
                        stats_back(pp, g, [cc], *fr.pop((pp, g, cc)))

            # both pairs' attention interleaved per t-block: the projections
            # for block c+1 are the PE filler under the ACT-bound attention
            for c in range(4):
                xqt = xq0 if c == 0 else load_xblock(xq_d, c, "xq")
                qk_block(xqt, wq_s, qT, 0, c)
                qk_block(xqt, wq_s, qT, 1, c)
                xkt = xk0 if c == 0 else load_xblock(xk_d, c, "xk")
                qk_block(xkt, wk_s, kT, 0, c)
                qk_block(xkt, wk_s, kT, 1, c)
                xvt = xv0 if c == 0 else load_xblock(xv_d, c, "xv")
                v_block(xvt, c)
                for pp in range(2):
                    for g in range(2):
                        attn_group(pp, c, g)
                        run_sched(sched.get(4 * c + 2 * pp + g, []))
            drain(0)
            run_sched(tail_sched)

            # ---- y = o_norm^T @ WoT (partial; host sums over cores) ----
            def y_chunk(tt):
                py = psum.tile([128, 2, 512], f32, tag="sc", bufs=2, name="py")
                for e in range(2):
                    for pp in range(2):
                        nc.tensor.matmul(
                            py[:, e, :],
                            o_norm[(pp, tt // 4)][
                                :, 128 * (tt % 4) : 128 * (tt % 4) + 128],
                            wo_s[:, pp, 512 * e : 512 * e + 512],
                            start=(pp == 0),
                            stop=(pp == 1),
                        )
                ys = ypool.tile([128, 2, 512], f16, tag="ys", name="ys")
                nc.vector.tensor_copy(ys[:, 0, :], py[:, 0, :])
                nc.scalar.copy(ys[:, 1, :], py[:, 1, :])
                nc.sync.dma_start(
                    out=y_d[128 * tt : 128 * tt + 128, :],
                    in_=ys.rearrange("p e t -> p (e t)"),
                )

            for tt in range(16):
                y_chunk(tt)

    nc.compile()
    return nc


def _prep(inputs):
    a = {k: np.asarray(v) for k, v in inputs.items()}
    lam = float(
        np.exp(np.sum(a["lq1"] * a["lk1"], dtype=np.float32))
        - np.exp(np.sum(a["lq2"] * a["lk2"], dtype=np.float32))
        + LAMBDA_INIT
    )
    wq_t = (a["Wq"].T / math.sqrt(HALF)).astype(np.float16)
    wk_t = a["Wk"].T.astype(np.float16)
    wv_t = a["Wv"].T.astype(np.float16)
    wo_g = (a["Wo"] * (np.tile(a["g"], H) * (1.0 - LAMBDA_INIT))[None, :]).T.astype(
        np.float16
    )
    r = np.arange(128)
    # ramp mask pair: (ma^T mb)[n, t] = -64*(n - t) for n > t else 0
    ma = (8.0 * (r[:, None] < r[None, :])).astype(np.float16)       # [d, n]
    mb = (-8.0 * (r[:, None] >= r[None, :])).astype(np.float16)     # [d, t]

    in_maps = []
    for core in range(N_CORES):
        b, hs = core // 4, 4 * (core % 4)
        sl = slice(DH * hs, DH * hs + DH * HPC)
        in_maps.append({
            "xq": np.ascontiguousarray(a["query"][b].T).astype(np.float16),
            "xk": np.ascontiguousarray(a["key_"][b].T).astype(np.float16),
            "xv": np.ascontiguousarray(a["value"][b].T).astype(np.float16),
            "wq": np.ascontiguousarray(wq_t[:, sl]),
            "wk": np.ascontiguousarray(wk_t[:, sl]),
            "wv": np.ascontiguousarray(wv_t[:, sl]),
            "wo": np.ascontiguousarray(wo_g[sl, :]),
            "ma": ma,
            "mb": mb,
        })
    return lam, in_maps


def run(inputs, trace=False, reps=1):
    lam, in_maps = _prep(inputs)
    key = (round(lam, 6), reps)
    if key not in _CACHE:
        _CACHE[key] = _build(lam, reps)
    nc = _CACHE[key]
    res = run_bass_kernel_spmd(
        nc, in_maps, core_ids=list(range(N_CORES)), trace=trace
    )
    out = np.empty((B, T, HID), np.float32)
    for b in range(B):
        out[b] = sum(res.results[4 * b + i]["y"].astype(np.float32) for i in range(4))
    return out, res


def kernel(**inputs) -> np.ndarray:
    out, _ = run(inputs)
    return out
